# revision 14
# baseline (speedup 1.0000x reference)
"""CrossCoder kernel for 8 Trainium2 NeuronCores (Bass/Tile, SPMD).

Math (reference):
    f     = relu(einsum('bld,ldf->bf', x, W_enc) + b_enc)     # [B, F]
    x_hat = einsum('bf,lfd->bld', f, W_dec) + b_dec           # [B, L, D]

Sharding: dict dim F=32768 split 8 ways (FL=4096 per core, tensor parallel
over latents). Each core computes its local f shard (encode) and the
partial decode sum over its latents; ReduceScatters combine the partials,
leaving each core with a distinct (ld-tile, batch-half) slice of the
transposed output, which the host reassembles.

Perf structure (PE sustains ~263ns per [128x128x512] matmul at the P0
sustained clock; 2048 MMs/core = ~538us is the floor):
- all matmul operands bf16 (FWL + LDWEIGHTS fully hidden), PSUM fp32
- weights streamed ONCE: each [128,512] weight tile feeds 8 matmuls
  (4 PE-column chunks x 2 batch halves) into all 8 PSUM banks
- x tiles DMA'd on the GpSimd queue and biases on the Scalar queue so the
  Sync queue's first transfer is the first weight tile (startup latency)
- decode partials/collectives in bf16; decode block (l,dg) ReduceScatters
  as soon as its 8 tiles land, overlapping the next block; the final block
  is split into two 4-bank sub-blocks (sharing one resident weight load)
  so only a 0.5MB RS piece remains exposed at the end
- evacuations alternate Vector/Scalar engines to halve the drain at block
  boundaries; b_dec/8 is folded in pre-collective
"""

import numpy as np
import ml_dtypes

B = 1024
L = 2
D = 1024
F = 32768
NCORES = 8
FL = F // NCORES      # 4096 latents per core
LD = L * D            # 2048
KT = LD // 128        # 16 encode k-tiles
FT = FL // 128        # 32 f-tiles per core
FG = FT // 4          # 8 encode f-groups (512 f-cols each)
NB = 512              # matmul moving free dim (PSUM bank capacity in fp32)
NH = 2                # batch halves

_CACHE = {}


def _build_nc():
    import concourse.bass as bass  # noqa: F401
    import concourse.tile as tile
    from concourse import bacc, mybir

    f32 = mybir.dt.float32
    bf16 = mybir.dt.bfloat16
    Relu = mybir.ActivationFunctionType.Relu
    Identity = mybir.ActivationFunctionType.Identity

    nc = bacc.Bacc()

    xT = nc.declare_dram_parameter("xT", [KT, 128, B], bf16, isOutput=False)
    w_enc = nc.declare_dram_parameter("w_enc", [FG, KT, 128, NB], bf16, isOutput=False)
    w_dec = nc.declare_dram_parameter("w_dec", [L, 2, FT, 128, NB], bf16, isOutput=False)
    b_enc = nc.declare_dram_parameter("b_enc", [128, FT], f32, isOutput=False)
    b_dec8 = nc.declare_dram_parameter("b_dec8", [128, KT], f32, isOutput=False)
    # out_sh[p], p = 2*l + dg in {0,1,2}: this core's RS shard of block (l,dg)
    # out_sh2[sp]: shards of the two split sub-blocks of (l=1, dg=1)
    out_sh = nc.declare_dram_parameter("out_sh", [3, 128, NB], bf16, isOutput=True)
    out_sh2 = nc.declare_dram_parameter("out_sh2", [2, 64, NB], bf16, isOutput=True)

    parts = [nc.dram_tensor(f"partial{p}", [8, 128, NB], bf16) for p in range(3)]
    rss = [nc.dram_tensor(f"rs{p}", [1, 128, NB], bf16) for p in range(3)]
    parts2 = [nc.dram_tensor(f"partial3{sp}", [4, 128, NB], bf16) for sp in range(2)]
    rss2 = [nc.dram_tensor(f"rs3{sp}", [64, NB], bf16) for sp in range(2)]

    xT_a = xT.ap()
    w_enc_a = w_enc.ap()
    w_dec_a = w_dec.ap()
    out_a = out_sh.ap()
    out2_a = out_sh2.ap()
    rgroups = [list(range(NCORES))]

    with tile.TileContext(nc) as tc:
        with (
            tc.tile_pool(name="xp", bufs=1) as xp,
            tc.tile_pool(name="fp", bufs=1) as fp,
            tc.tile_pool(name="we", bufs=12) as we,
            tc.tile_pool(name="wd", bufs=12) as wd,
            tc.tile_pool(name="wdl", bufs=1) as wdl,
            tc.tile_pool(name="stg", bufs=8) as stg,
            tc.tile_pool(name="bias", bufs=1) as bias,
            tc.tile_pool(name="ps", bufs=8, space="PSUM") as ps,
        ):
            # biases on the Scalar DMA queue, x on GpSimd: the Sync queue's
            # first transfer is then w_enc[0,0]
            benc_t = bias.tile([128, FT], f32, name="benc")
            nc.scalar.dma_start(out=benc_t, in_=b_enc.ap())
            bdec_t = bias.tile([128, KT], f32, name="bdec")
            nc.scalar.dma_start(out=bdec_t, in_=b_dec8.ap())

            x_tiles = []
            for k in range(KT):
                xt = xp.tile([128, B], bf16, tag=f"x{k}", name=f"x{k}")
                nc.gpsimd.dma_start(out=xt, in_=xT_a[k])
                x_tiles.append(xt)

            # ---- encode: f[h] = relu(xT.T @ W_enc + b_enc), both halves
            # per weight tile load
            f_tiles = []  # [fk][h]
            for fg in range(FG):
                pss = [
                    ps.tile([128, NB], f32, tag="ps", name=f"pse{_j}")
                    for _j in range(8)
                ]
                for k in range(KT):
                    wt = we.tile([128, NB], bf16, tag="we", name="wet")
                    nc.sync.dma_start(out=wt, in_=w_enc_a[fg, k])
                    for j in range(4):
                        for h in range(NH):
                            nc.tensor.matmul(
                                pss[j * 2 + h],
                                wt[:, j * 128 : (j + 1) * 128],
                                x_tiles[k][:, h * NB : (h + 1) * NB],
                                start=(k == 0),
                                stop=(k == KT - 1),
                            )
                for j in range(4):
                    ft_idx = fg * 4 + j
                    pair = []
                    for h in range(NH):
                        ftile = fp.tile(
                            [128, NB], bf16, tag=f"f{ft_idx}_{h}", name=f"f{ft_idx}_{h}"
                        )
                        nc.scalar.activation(
                            ftile,
                            pss[j * 2 + h],
                            Relu,
                            bias=benc_t[:, ft_idx : ft_idx + 1],
                        )
                        pair.append(ftile)
                    f_tiles.append(pair)

            def evac(pst, ld_t, h, out_ap):
                """PSUM -> (+ b_dec/8) -> bf16 staging -> DRAM partial.

                Compute engine and DMA queue alternate with h so the drain
                at a block boundary runs two-wide on both resources.
                """
                st = stg.tile([128, NB], bf16, tag="st", name="st")
                if h == 0:
                    nc.vector.tensor_scalar_add(
                        st, pst, bdec_t[:, ld_t : ld_t + 1]
                    )
                    nc.sync.dma_start(out=out_ap, in_=st)
                else:
                    nc.scalar.activation(
                        st, pst, Identity, bias=bdec_t[:, ld_t : ld_t + 1]
                    )
                    nc.scalar.dma_start(out=out_ap, in_=st)

            # ---- decode: partial[ld, b] = W_dec.T @ f, both halves per load
            for l in range(L):
                for dg in range(2):
                    p = l * 2 + dg
                    if p < 3:
                        pss = [
                            ps.tile([128, NB], f32, tag="ps", name=f"psd{_j}")
                            for _j in range(8)
                        ]
                        for fk in range(FT):
                            wt = wd.tile([128, NB], bf16, tag="wd", name="wdt")
                            nc.sync.dma_start(out=wt, in_=w_dec_a[l, dg, fk])
                            for j in range(4):
                                for h in range(NH):
                                    nc.tensor.matmul(
                                        pss[j * 2 + h],
                                        wt[:, j * 128 : (j + 1) * 128],
                                        f_tiles[fk][h],
                                        start=(fk == 0),
                                        stop=(fk == FT - 1),
                                    )
                        part_a = parts[p].ap()
                        for j in range(4):
                            ld_t = l * 8 + dg * 4 + j
                            for h in range(NH):
                                evac(pss[j * 2 + h], ld_t, h, part_a[j * 2 + h])
                        nc.gpsimd.collective_compute(
                            "ReduceScatter",
                            mybir.AluOpType.add,
                            ins=[parts[p][:]],
                            outs=[rss[p][:]],
                            replica_groups=rgroups,
                        )
                        nc.gpsimd.dma_start(out=out_a[p : p + 1], in_=rss[p][:])
                    else:
                        # final block: two 4-bank sub-blocks over a single
                        # resident weight load; each fires its own small RS
                        wl_tiles = []
                        for fk in range(FT):
                            wt = wdl.tile(
                                [128, NB], bf16, tag=f"wl{fk}", name=f"wl{fk}"
                            )
                            nc.sync.dma_start(out=wt, in_=w_dec_a[l, dg, fk])
                            wl_tiles.append(wt)
                        for sp in range(2):
                            pss = [
                                ps.tile([128, NB], f32, tag="ps", name=f"pss{_j}")
                                for _j in range(4)
                            ]
                            for fk in range(FT):
                                for jl in range(2):
                                    j = sp * 2 + jl
                                    for h in range(NH):
                                        nc.tensor.matmul(
                                            pss[jl * 2 + h],
                                            wl_tiles[fk][:, j * 128 : (j + 1) * 128],
                                            f_tiles[fk][h],
                                            start=(fk == 0),
                                            stop=(fk == FT - 1),
                                        )
                            part_a = parts2[sp].ap()
                            for jl in range(2):
                                j = sp * 2 + jl
                                ld_t = l * 8 + dg * 4 + j
                                for h in range(NH):
                                    evac(pss[jl * 2 + h], ld_t, h, part_a[jl * 2 + h])
                            nc.gpsimd.collective_compute(
                                "ReduceScatter",
                                mybir.AluOpType.add,
                                ins=[parts2[sp][:]],
                                outs=[rss2[sp][:]],
                                replica_groups=rgroups,
                            )
                            nc.gpsimd.dma_start(out=out2_a[sp], in_=rss2[sp][:])

    nc.finalize()
    return nc


def _get_nc():
    if "nc" not in _CACHE:
        _CACHE["nc"] = _build_nc()
    return _CACHE["nc"]


def kernel(x, W_enc, b_enc, W_dec, b_dec):
    from concourse.bass_utils import run_bass_kernel_spmd

    x = np.asarray(x, dtype=np.float32)
    W_enc = np.asarray(W_enc, dtype=np.float32)
    b_enc = np.asarray(b_enc, dtype=np.float32)
    W_dec = np.asarray(W_dec, dtype=np.float32)
    b_dec = np.asarray(b_dec, dtype=np.float32)

    nc = _get_nc()

    bf = ml_dtypes.bfloat16
    # xT row k*128+p (= x.reshape(B,LD).T), col b
    xT = np.ascontiguousarray(x.reshape(B, LD).T.reshape(KT, 128, B)).astype(bf)
    w_enc_flat = W_enc.reshape(LD, F)
    bdec8 = np.ascontiguousarray(
        (b_dec.reshape(LD) / NCORES).astype(np.float32).reshape(KT, 128).T
    )

    in_maps = []
    for i in range(NCORES):
        fsl = slice(i * FL, (i + 1) * FL)
        we_blk = (
            w_enc_flat[:, fsl].reshape(KT, 128, FG, NB).transpose(2, 0, 1, 3)
        ).astype(bf)
        wd_blk = (
            W_dec[:, fsl, :].reshape(L, FT, 128, 2, NB).transpose(0, 3, 1, 2, 4)
        ).astype(bf)
        in_maps.append(
            {
                "xT": xT,
                "w_enc": np.ascontiguousarray(we_blk),
                "w_dec": np.ascontiguousarray(wd_blk),
                "b_enc": np.ascontiguousarray(b_enc[fsl].reshape(FT, 128).T),
                "b_dec8": bdec8,
            }
        )

    res = run_bass_kernel_spmd(nc, in_maps, list(range(NCORES)))
    _CACHE["last_res"] = res

    xhatT = np.empty((LD, B), dtype=np.float32)
    for i in range(NCORES):
        arr3 = np.asarray(res.results[i]["out_sh"]).astype(np.float32)
        arr2 = np.asarray(res.results[i]["out_sh2"]).astype(np.float32)
        j, h = i // 2, i % 2
        for p in range(3):
            l, dg = p // 2, p % 2
            r0 = 128 * (l * 8 + dg * 4 + j)
            xhatT[r0 : r0 + 128, h * NB : (h + 1) * NB] = arr3[p]
        # sub-blocks of (l=1, dg=1): core i holds tile t=i//2 rows (i%2)*64..
        jl, h2, rh = (i // 2) // 2, (i // 2) % 2, i % 2
        for sp in range(2):
            r0 = 128 * (12 + sp * 2 + jl) + rh * 64
            xhatT[r0 : r0 + 64, h2 * NB : (h2 + 1) * NB] = arr2[sp]
    return np.ascontiguousarray(xhatT.T).reshape(B, L, D).astype(np.float32)


# revision 19
# speedup vs baseline: 1.0220x; 1.0220x over previous
"""CrossCoder kernel for 8 Trainium2 NeuronCores (Bass/Tile, SPMD).

Math (reference):
    f     = relu(einsum('bld,ldf->bf', x, W_enc) + b_enc)     # [B, F]
    x_hat = einsum('bf,lfd->bld', f, W_dec) + b_dec           # [B, L, D]

Sharding: dict dim F=32768 split 8 ways (FL=4096 per core, tensor parallel
over latents). Each core computes its local f shard (encode) and the
partial decode sum over its latents; ReduceScatters combine the partials,
leaving each core with a distinct (ld-tile, batch-half) slice of the
transposed output, which the host reassembles.

Perf structure (PE sustains ~263ns per [128x128x512] matmul at the P0
sustained clock; 2048 MMs/core = ~538us is the floor):
- all matmul operands bf16 (FWL + LDWEIGHTS fully hidden), PSUM fp32
- weights streamed ONCE: each [128,512] weight tile feeds 8 matmuls
  (4 PE-column chunks x 2 batch halves) into all 8 PSUM banks
- x tiles DMA'd on the GpSimd queue and biases on the Scalar queue so the
  Sync queue's first transfer is the first weight tile (startup latency)
- decode partials/collectives in bf16; decode block (l,dg) ReduceScatters
  as soon as its 8 tiles land, overlapping the next block; the final block
  is split into two 4-bank sub-blocks (sharing one resident weight load)
  so only a 0.5MB RS piece remains exposed at the end
- evacuations alternate Vector/Scalar engines to halve the drain at block
  boundaries; b_dec/8 is folded in pre-collective
"""

import numpy as np
import ml_dtypes

B = 1024
L = 2
D = 1024
F = 32768
NCORES = 8
FL = F // NCORES      # 4096 latents per core
LD = L * D            # 2048
KT = LD // 128        # 16 encode k-tiles
FT = FL // 128        # 32 f-tiles per core
FG = FT // 4          # 8 encode f-groups (512 f-cols each)
NB = 512              # matmul moving free dim (PSUM bank capacity in fp32)
NH = 2                # batch halves

_CACHE = {}


def _build_nc():
    import concourse.bass as bass  # noqa: F401
    import concourse.tile as tile
    from concourse import bacc, mybir

    f32 = mybir.dt.float32
    bf16 = mybir.dt.bfloat16
    Relu = mybir.ActivationFunctionType.Relu
    Identity = mybir.ActivationFunctionType.Identity

    nc = bacc.Bacc()

    xT = nc.declare_dram_parameter("xT", [KT, 128, B], bf16, isOutput=False)
    w_enc = nc.declare_dram_parameter("w_enc", [FG, KT, 128, NB], bf16, isOutput=False)
    w_dec = nc.declare_dram_parameter("w_dec", [L, 2, FT, 128, NB], bf16, isOutput=False)
    b_enc = nc.declare_dram_parameter("b_enc", [128, FT], f32, isOutput=False)
    b_dec8 = nc.declare_dram_parameter("b_dec8", [128, KT], f32, isOutput=False)
    # out_sh[p], p = 2*l + dg in {0,1,2}: this core's RS shard of block (l,dg)
    # out_sh2[sp]: shards of the two split sub-blocks of (l=1, dg=1)
    out_sh = nc.declare_dram_parameter("out_sh", [3, 128, NB], bf16, isOutput=True)
    out_sh2 = nc.declare_dram_parameter("out_sh2", [2, 64, NB], bf16, isOutput=True)

    parts = [nc.dram_tensor(f"partial{p}", [8, 128, NB], bf16) for p in range(3)]
    rss = [nc.dram_tensor(f"rs{p}", [1, 128, NB], bf16) for p in range(3)]
    parts2 = [nc.dram_tensor(f"partial3{sp}", [4, 128, NB], bf16) for sp in range(2)]
    rss2 = [nc.dram_tensor(f"rs3{sp}", [64, NB], bf16) for sp in range(2)]

    xT_a = xT.ap()
    w_enc_a = w_enc.ap()
    w_dec_a = w_dec.ap()
    out_a = out_sh.ap()
    out2_a = out_sh2.ap()
    rgroups = [list(range(NCORES))]

    with tile.TileContext(nc) as tc:
        with (
            tc.tile_pool(name="xp", bufs=1) as xp,
            tc.tile_pool(name="fp", bufs=1) as fp,
            tc.tile_pool(name="we", bufs=12) as we,
            tc.tile_pool(name="wd", bufs=12) as wd,
            tc.tile_pool(name="wdl", bufs=1) as wdl,
            tc.tile_pool(name="stg", bufs=8) as stg,
            tc.tile_pool(name="bias", bufs=1) as bias,
            tc.tile_pool(name="ps", bufs=8, space="PSUM") as ps,
        ):
            # biases on the Scalar DMA queue, x on GpSimd: the Sync queue's
            # first transfer is then w_enc[0,0]
            benc_t = bias.tile([128, FT], f32, name="benc")
            nc.scalar.dma_start(out=benc_t, in_=b_enc.ap())
            bdec_t = bias.tile([128, KT], f32, name="bdec")
            nc.scalar.dma_start(out=bdec_t, in_=b_dec8.ap())

            # x0 rides first on the Sync queue (shortest preamble) since it
            # gates the first matmul; the rest stream on GpSimd in parallel
            x_tiles = []
            for k in range(KT):
                xt = xp.tile([128, B], bf16, tag=f"x{k}", name=f"x{k}")
                if k == 0:
                    nc.sync.dma_start(out=xt, in_=xT_a[k])
                else:
                    nc.gpsimd.dma_start(out=xt, in_=xT_a[k])
                x_tiles.append(xt)

            # ---- encode: f[h] = relu(xT.T @ W_enc + b_enc), both halves
            # per weight tile load
            f_tiles = []  # [fk][h]
            for fg in range(FG):
                pss = [
                    ps.tile([128, NB], f32, tag="ps", name=f"pse{_j}")
                    for _j in range(8)
                ]
                for k in range(KT):
                    wt = we.tile([128, NB], bf16, tag="we", name="wet")
                    nc.sync.dma_start(out=wt, in_=w_enc_a[fg, k])
                    for j in range(4):
                        for h in range(NH):
                            nc.tensor.matmul(
                                pss[j * 2 + h],
                                wt[:, j * 128 : (j + 1) * 128],
                                x_tiles[k][:, h * NB : (h + 1) * NB],
                                start=(k == 0),
                                stop=(k == KT - 1),
                            )
                for j in range(4):
                    ft_idx = fg * 4 + j
                    pair = []
                    for h in range(NH):
                        ftile = fp.tile(
                            [128, NB], bf16, tag=f"f{ft_idx}_{h}", name=f"f{ft_idx}_{h}"
                        )
                        nc.scalar.activation(
                            ftile,
                            pss[j * 2 + h],
                            Relu,
                            bias=benc_t[:, ft_idx : ft_idx + 1],
                        )
                        pair.append(ftile)
                    f_tiles.append(pair)

            def evac(pst, ld_t, h, out_ap):
                """PSUM -> (+ b_dec/8) -> bf16 staging -> DRAM partial.

                Compute engine and DMA queue alternate with h so the drain
                at a block boundary runs two-wide on both resources.
                """
                st = stg.tile([128, NB], bf16, tag="st", name="st")
                if h == 0:
                    nc.vector.tensor_scalar_add(
                        st, pst, bdec_t[:, ld_t : ld_t + 1]
                    )
                    nc.sync.dma_start(out=out_ap, in_=st)
                else:
                    nc.scalar.activation(
                        st, pst, Identity, bias=bdec_t[:, ld_t : ld_t + 1]
                    )
                    nc.scalar.dma_start(out=out_ap, in_=st)

            # ---- decode: partial[ld, b] = W_dec.T @ f, both halves per load
            for l in range(L):
                for dg in range(2):
                    p = l * 2 + dg
                    if p < 3:
                        pss = [
                            ps.tile([128, NB], f32, tag="ps", name=f"psd{_j}")
                            for _j in range(8)
                        ]
                        for fk in range(FT):
                            wt = wd.tile([128, NB], bf16, tag="wd", name="wdt")
                            nc.sync.dma_start(out=wt, in_=w_dec_a[l, dg, fk])
                            for j in range(4):
                                for h in range(NH):
                                    nc.tensor.matmul(
                                        pss[j * 2 + h],
                                        wt[:, j * 128 : (j + 1) * 128],
                                        f_tiles[fk][h],
                                        start=(fk == 0),
                                        stop=(fk == FT - 1),
                                    )
                        part_a = parts[p].ap()
                        for j in range(4):
                            ld_t = l * 8 + dg * 4 + j
                            for h in range(NH):
                                evac(pss[j * 2 + h], ld_t, h, part_a[j * 2 + h])
                        nc.gpsimd.collective_compute(
                            "ReduceScatter",
                            mybir.AluOpType.add,
                            ins=[parts[p][:]],
                            outs=[rss[p][:]],
                            replica_groups=rgroups,
                        )
                        nc.gpsimd.dma_start(out=out_a[p : p + 1], in_=rss[p][:])
                    else:
                        # final block: two 4-bank sub-blocks over a single
                        # resident weight load; each fires its own small RS
                        wl_tiles = []
                        for fk in range(FT):
                            wt = wdl.tile(
                                [128, NB], bf16, tag=f"wl{fk}", name=f"wl{fk}"
                            )
                            nc.sync.dma_start(out=wt, in_=w_dec_a[l, dg, fk])
                            wl_tiles.append(wt)
                        for sp in range(2):
                            pss = [
                                ps.tile([128, NB], f32, tag="ps", name=f"pss{_j}")
                                for _j in range(4)
                            ]
                            for fk in range(FT):
                                for jl in range(2):
                                    j = sp * 2 + jl
                                    for h in range(NH):
                                        nc.tensor.matmul(
                                            pss[jl * 2 + h],
                                            wl_tiles[fk][:, j * 128 : (j + 1) * 128],
                                            f_tiles[fk][h],
                                            start=(fk == 0),
                                            stop=(fk == FT - 1),
                                        )
                            part_a = parts2[sp].ap()
                            for jl in range(2):
                                j = sp * 2 + jl
                                ld_t = l * 8 + dg * 4 + j
                                for h in range(NH):
                                    evac(pss[jl * 2 + h], ld_t, h, part_a[jl * 2 + h])
                            nc.gpsimd.collective_compute(
                                "ReduceScatter",
                                mybir.AluOpType.add,
                                ins=[parts2[sp][:]],
                                outs=[rss2[sp][:]],
                                replica_groups=rgroups,
                            )
                            nc.gpsimd.dma_start(out=out2_a[sp], in_=rss2[sp][:])

    nc.finalize()
    return nc


def _get_nc():
    if "nc" not in _CACHE:
        _CACHE["nc"] = _build_nc()
    return _CACHE["nc"]


def kernel(x, W_enc, b_enc, W_dec, b_dec):
    from concourse.bass_utils import run_bass_kernel_spmd

    x = np.asarray(x, dtype=np.float32)
    W_enc = np.asarray(W_enc, dtype=np.float32)
    b_enc = np.asarray(b_enc, dtype=np.float32)
    W_dec = np.asarray(W_dec, dtype=np.float32)
    b_dec = np.asarray(b_dec, dtype=np.float32)

    nc = _get_nc()

    bf = ml_dtypes.bfloat16
    # xT row k*128+p (= x.reshape(B,LD).T), col b
    xT = np.ascontiguousarray(x.reshape(B, LD).T.reshape(KT, 128, B)).astype(bf)
    w_enc_flat = W_enc.reshape(LD, F)
    bdec8 = np.ascontiguousarray(
        (b_dec.reshape(LD) / NCORES).astype(np.float32).reshape(KT, 128).T
    )

    in_maps = []
    for i in range(NCORES):
        fsl = slice(i * FL, (i + 1) * FL)
        we_blk = (
            w_enc_flat[:, fsl].reshape(KT, 128, FG, NB).transpose(2, 0, 1, 3)
        ).astype(bf)
        wd_blk = (
            W_dec[:, fsl, :].reshape(L, FT, 128, 2, NB).transpose(0, 3, 1, 2, 4)
        ).astype(bf)
        in_maps.append(
            {
                "xT": xT,
                "w_enc": np.ascontiguousarray(we_blk),
                "w_dec": np.ascontiguousarray(wd_blk),
                "b_enc": np.ascontiguousarray(b_enc[fsl].reshape(FT, 128).T),
                "b_dec8": bdec8,
            }
        )

    res = run_bass_kernel_spmd(nc, in_maps, list(range(NCORES)))
    _CACHE["last_res"] = res

    xhatT = np.empty((LD, B), dtype=np.float32)
    for i in range(NCORES):
        arr3 = np.asarray(res.results[i]["out_sh"]).astype(np.float32)
        arr2 = np.asarray(res.results[i]["out_sh2"]).astype(np.float32)
        j, h = i // 2, i % 2
        for p in range(3):
            l, dg = p // 2, p % 2
            r0 = 128 * (l * 8 + dg * 4 + j)
            xhatT[r0 : r0 + 128, h * NB : (h + 1) * NB] = arr3[p]
        # sub-blocks of (l=1, dg=1): core i holds tile t=i//2 rows (i%2)*64..
        jl, h2, rh = (i // 2) // 2, (i // 2) % 2, i % 2
        for sp in range(2):
            r0 = 128 * (12 + sp * 2 + jl) + rh * 64
            xhatT[r0 : r0 + 64, h2 * NB : (h2 + 1) * NB] = arr2[sp]
    return np.ascontiguousarray(xhatT.T).reshape(B, L, D).astype(np.float32)
